# revision 1
# baseline (speedup 1.0000x reference)
"""Causal multi-head attention block (B=2, S=2048, D=768, H=12) on 8 trn2 cores.

Sharding: core c -> batch b = c//4 (data parallel), head group g = c%4
(tensor parallel, 3 heads per group). Each core computes its group's QKV
projection, causal attention, and a partial O-projection over its 192
z-columns. Host sums the 4 partials per batch and adds the biases that
commute through the math (v-bias and b_o).

Layouts: xT [768, 2048] (d on partitions) so q^T/k^T come straight out of
the projection with head dims on partitions; V is projected directly in
[keys, dh] layout (lhsT = x key-block, rhs = W_v^T columns) so attention
needs no on-chip transposes. A ones-column appended to V makes the PV
matmul emit the softmax denominator for free.

Projections run as residual-compensated fp8 DoubleRow matmuls (0.5
cycles/row): host pre-scales x and W by 32, splits each into fp8 main m
and fp8 residual r = fp8(a - m), and the kernel accumulates m.m + m.rx +
rW.m into psum (error ~2e-3 overall); evictions descale by 1/1024 and add
the q/k biases. Attention (scores, PV) and the O-projection stay fp16
(1 cycle/row, ~1e-4-level error). Scores for kt pairs land in one 2-bank
psum tile so a single exp covers both; start=True zeroing covers the
unwritten low columns of narrowed diagonal halves (exp to 1.0, never
consumed); the causal mask only touches the 128-wide diagonal square on
the gpsimd engine. Softmax normalization runs reciprocal (DVE) ->
partition_broadcast (gpsimd) -> multiply (DVE), keeping the PE out of it.

The packed q/k projection weight is host-repacked so all three 128-wide
M-groups are full and each head's q/k share a partition base:
  m0=[q_h0 q_h1] -> qT col0,  m1=[k_h0 k_h1] -> kT col0,
  m2=[q_h2 k_h2] -> qT/kT col1 (k2 rows evict across partition bases).
(q rows pre-scaled by 1/8; v bias folded into the host-side epilogue.)

Scheduling: projection/V/O-proj work lives in a background queue drained
between attention kt-pairs so the scalar engine (exp) is never starved;
O-proj for the first two q-tiles is deferred into the exp-dense last
q-tile. The final block accumulates PV into per-128-column psum groups
(closed at kt = nkt-4+q) and pipelines normalize -> O-proj -> DMA per
column quarter to shrink the epilogue tail.
"""

import os
from collections import deque
from contextlib import ExitStack

import numpy as np

import concourse.tile as tile
from concourse import bacc, mybir
from concourse.bass_utils import run_bass_kernel_spmd

F32 = mybir.dt.float32
F16 = mybir.dt.float16
F8 = mybir.dt.float8e4
DR = mybir.MatmulPerfMode.DoubleRow
AF = mybir.ActivationFunctionType

B, S, D = 2, 2048, 768
NH, DH = 12, 64
HPC = 3            # heads per core
GD = HPC * DH      # 192 z-cols per core
KT, QT = 128, 512  # key tile (partitions), q tile (psum free)
NKT, NQT = S // KT, S // QT   # 16, 4
NKD = D // 128     # 6 contraction tiles for the projections
WPK = 2 * GD       # 384 packed q/k projection rows


def build_bass():
    nc = bacc.Bacc(None)
    x8 = nc.dram_tensor("x8", [D, S], F8, kind="ExternalInput")
    xr = nc.dram_tensor("xr", [D, S], F8, kind="ExternalInput")
    wpk8 = nc.dram_tensor("wpk8", [D, WPK], F8, kind="ExternalInput")
    wpkr = nc.dram_tensor("wpkr", [D, WPK], F8, kind="ExternalInput")
    wv8 = nc.dram_tensor("wv8", [D, GD], F8, kind="ExternalInput")
    wvr = nc.dram_tensor("wvr", [D, GD], F8, kind="ExternalInput")
    woT = nc.dram_tensor("woT", [GD, D], F16, kind="ExternalInput")
    bqk = nc.dram_tensor("bqk", [128, 3], F32, kind="ExternalInput")
    vones = nc.dram_tensor("vones", [128, 64], F16, kind="ExternalInput")
    out_p = nc.dram_tensor("out_p", [S, D], F16, kind="ExternalOutput")

    with tile.TileContext(nc) as tc, ExitStack() as ctx:
        const = ctx.enter_context(tc.tile_pool(name="const", bufs=1))
        ps = ctx.enter_context(tc.tile_pool(name="ps", bufs=2, space="PSUM"))
        ps2 = ctx.enter_context(tc.tile_pool(name="ps2", bufs=2, space="PSUM"))
        psz = ctx.enter_context(tc.tile_pool(name="psz", bufs=2, space="PSUM"))
        expp = ctx.enter_context(tc.tile_pool(name="expp", bufs=int(os.environ.get("KEXP", "24"))))
        small = ctx.enter_context(tc.tile_pool(name="small", bufs=int(os.environ.get("KSM", "6"))))

        x8_sb = const.tile([128, 3, 2, S], F8)
        xr_sb = const.tile([128, 3, 2, S], F8)
        wpk8_sb = const.tile([128, 3, 2, WPK], F8)
        wpkr_sb = const.tile([128, 3, 2, WPK], F8)
        wv8_sb = const.tile([128, 3, 2, GD], F8)
        wvr_sb = const.tile([128, 3, 2, GD], F8)
        wo_a = const.tile([128, D], F16)
        wo_b = const.tile([64, D], F16)
        bqk_sb = const.tile([128, 3], F32)
        qT_sb = const.tile([128, 2, S], F16)
        kT_sb = const.tile([128, 2, S], F16)
        v_aug = const.tile([128, NKT, HPC, DH + 1], F16)
        zT01 = const.tile([128, S], F16)
        zT2 = const.tile([64, S], F16)
        ones_stage = const.tile([128, 64], F16)

        # ---- loads: k-interleaved so the first projection k-pairs unblock
        # early. Early DMAs fan out over four queues (SP/Act/DVE/Pool) to
        # dodge the ~650ns per-DMA dispatch serialization on a single queue.
        # batched multi-tile DMAs keep the dispatch count low (each dispatch
        # costs ~650ns of queue time). The projection runs its main fp8 pass
        # first, so the x8/w8 arrays go first and the residual arrays follow.
        def tri(dram):
            return dram.rearrange("(kp two p) s -> p kp two s", p=128, two=2)

        x8_a, xr_a = tri(x8), tri(xr)
        wp8_a, wpr_a = tri(wpk8), tri(wpkr)
        wv8_a, wvr_a = tri(wv8), tri(wvr)
        # the first q-tile's operands stream first: weight slivers for m0,
        # then x8/xr column chunks; residual weights chase the mains.
        nc.sync.dma_start(out=wpk8_sb[:, :, :, 0:256], in_=wp8_a[:, :, :, 0:256])
        nc.scalar.dma_start(out=x8_sb[:, :, :, 0:QT], in_=x8_a[:, :, :, 0:QT])
        nc.sync.dma_start(out=xr_sb[:, :, :, 0:QT], in_=xr_a[:, :, :, 0:QT])
        nc.scalar.dma_start(out=wpkr_sb[:, :, :, 0:256], in_=wpr_a[:, :, :, 0:256])
        nc.sync.dma_start(out=bqk_sb[:], in_=bqk[:, :])
        nc.scalar.dma_start(
            out=wpk8_sb[:, :, :, 256:WPK], in_=wp8_a[:, :, :, 256:WPK]
        )
        nc.sync.dma_start(
            out=wpkr_sb[:, :, :, 256:WPK], in_=wpr_a[:, :, :, 256:WPK]
        )
        nc.gpsimd.dma_start(out=wv8_sb[:], in_=wv8_a[:])
        nc.gpsimd.dma_start(out=wvr_sb[:], in_=wvr_a[:])
        nc.gpsimd.dma_start(out=ones_stage[:], in_=vones[:, :])
        nc.vector.tensor_copy(
            out=v_aug[:, :, :, DH],
            in_=ones_stage[:, 0 : NKT * HPC].rearrange("p (t h) -> p t h", t=NKT),
        )
        nc.scalar.dma_start(
            out=x8_sb[:, :, :, QT : 2 * QT], in_=x8_a[:, :, :, QT : 2 * QT]
        )
        nc.sync.dma_start(
            out=xr_sb[:, :, :, QT : 2 * QT], in_=xr_a[:, :, :, QT : 2 * QT]
        )
        nc.scalar.dma_start(
            out=x8_sb[:, :, :, 2 * QT : S], in_=x8_a[:, :, :, 2 * QT : S]
        )
        nc.sync.dma_start(
            out=xr_sb[:, :, :, 2 * QT : S], in_=xr_a[:, :, :, 2 * QT : S]
        )
        nc.sync.dma_start(out=wo_a[:], in_=woT[0:128, :])
        nc.gpsimd.dma_start(out=wo_b[:], in_=woT[128:GD, :])

        # packed q/k projection m-groups: (col0, evict spec). m2 holds
        # [q2 k2]; its k2 rows evict to kT partitions 0:64 (cross-base copy)
        # so every head's q and k share a partition base for the PE.
        mgroups = [
            (0, [((0, 128), lambda n: qT_sb[0:128, 0, n * QT : (n + 1) * QT], 0)]),
            (128, [((0, 128), lambda n: kT_sb[0:128, 0, n * QT : (n + 1) * QT], 1)]),
            (256, [
                ((0, 64), lambda n: qT_sb[0:64, 1, n * QT : (n + 1) * QT], 2),
                ((64, 128), lambda n: kT_sb[0:64, 1, n * QT : (n + 1) * QT], 2),
            ]),
        ]

        proj_psums = {}
        PROJ_TERMS = [(wpk8_sb, x8_sb), (wpk8_sb, xr_sb), (wpkr_sb, x8_sb)]
        DESCALE = 1.0 / 1024.0  # host pre-scales x and W by 32 for fp8 range

        def proj_unit(mi, n, term, use_ps2=False):
            """One fp8 DoubleRow pass (3 k-pairs) of group (mi, n): term 0 is
            the w8.x8 main product, terms 1/2 add the x and W residuals."""
            c0, evicts = mgroups[mi]
            key = (mi, n)
            if key not in proj_psums:
                if use_ps2:  # scores pool is idle during the prologue
                    p2 = ps2.tile([128, 2, QT], F32, tag="ps2", name="projp2")
                    proj_psums[key] = p2[:, 0]
                else:
                    proj_psums[key] = ps.tile([128, QT], F32, tag="ps", name="projp")
            p = proj_psums[key]
            wsb, xsb = PROJ_TERMS[term]
            for kp in range(3):
                nc.tensor.matmul(
                    p[:, :],
                    lhsT=wsb[:, kp, :, c0 : c0 + 128],
                    rhs=xsb[:, kp, :, n * QT : (n + 1) * QT],
                    start=(term == 0 and kp == 0),
                    stop=(term == 2 and kp == 2),
                    perf_mode=DR,
                )
            if term == 2:
                del proj_psums[key]
                for (r0, r1), dst, bcol in evicts:
                    if n < int(os.environ.get("KPEA", "0")):
                        nc.scalar.activation(
                            out=dst(n),
                            in_=p[r0:r1, :],
                            func=AF.Identity,
                            bias=bqk_sb[r0:r1, bcol : bcol + 1],
                            scale=DESCALE,
                        )
                    else:
                        nc.vector.tensor_scalar(
                            out=dst(n),
                            in0=p[r0:r1, :],
                            scalar1=DESCALE,
                            scalar2=bqk_sb[r0:r1, bcol : bcol + 1],
                            op0=mybir.AluOpType.mult,
                            op1=mybir.AluOpType.add,
                        )

        vp_psums = {}
        V_TERMS = [(x8_sb, wv8_sb), (xr_sb, wv8_sb), (x8_sb, wvr_sb)]

        def v_unit(t, term):
            """V projection for keys-tile t (fp8 DoubleRow pass `term`):
            [128 keys, 192] psum; evict into v_aug[:, t, :, 0:64]."""
            if t not in vp_psums:
                vp_psums[t] = ps.tile([128, QT], F32, tag="ps", name="vp")
            p = vp_psums[t]
            xsb, wsb = V_TERMS[term]
            for kp in range(3):
                nc.tensor.matmul(
                    p[:, 0:GD],
                    lhsT=xsb[:, kp, :, t * 128 : (t + 1) * 128],
                    rhs=wsb[:, kp, :, :],
                    start=(term == 0 and kp == 0),
                    stop=(term == 2 and kp == 2),
                    perf_mode=DR,
                )
            if term == 2:
                del vp_psums[t]
                if t < int(os.environ.get("KVEA", "10")):
                    nc.scalar.activation(
                        out=v_aug[:, t, :, 0:DH],
                        in_=p[:, 0:GD].rearrange("p (h d) -> p h d", h=HPC),
                        func=AF.Copy,
                        scale=DESCALE,
                    )
                else:
                    nc.vector.tensor_scalar_mul(
                        out=v_aug[:, t, :, 0:DH],
                        in0=p[:, 0:GD].rearrange("p (h d) -> p h d", h=HPC),
                        scalar1=DESCALE,
                    )

        out_t = out_p.rearrange("(tp p) d -> tp p d", p=128)
        o_done = {}

        def o_proj_unit(t, n2, evict_eng=None, tail=False):
            key = t
            if key not in o_done:
                o_done[key] = expp.tile([128, D], F16, tag="osb", name="osb", bufs=int(os.environ.get("KOSB", "3")))
            ob = o_done[key]
            po = ps.tile([128, QT], F32, tag="ps")
            nc.tensor.matmul(
                po[:, 0:384],
                lhsT=zT01[:, t * 128 : (t + 1) * 128],
                rhs=wo_a[:, n2 * 384 : (n2 + 1) * 384],
                start=True,
                stop=False,
            )
            nc.tensor.matmul(
                po[:, 0:384],
                lhsT=zT2[:, t * 128 : (t + 1) * 128],
                rhs=wo_b[:, n2 * 384 : (n2 + 1) * 384],
                start=False,
                stop=True,
            )
            eng = evict_eng
            if eng is None:
                eng = "dve"
            if eng == "act":
                nc.scalar.activation(
                    out=ob[:, n2 * 384 : (n2 + 1) * 384], in_=po[:, 0:384], func=AF.Copy
                )
            else:
                nc.vector.tensor_copy(
                    out=ob[:, n2 * 384 : (n2 + 1) * 384], in_=po[:, 0:384]
                )
            if tail:
                (nc.sync if n2 == 0 else nc.scalar).dma_start(
                    out=out_t[t][:, n2 * 384 : (n2 + 1) * 384],
                    in_=ob[:, n2 * 384 : (n2 + 1) * 384],
                )
                if n2 == 1:
                    del o_done[key]
            elif n2 == 1:
                del o_done[key]
                qsel = os.environ.get("KOQ", "sc")
                qmap = {"s": nc.sync, "c": nc.scalar, "g": nc.gpsimd}
                qmap[qsel[t % len(qsel)]].dma_start(out=out_t[t], in_=ob[:, :])

        # background work queue of (key, fn), drained between attention
        # iterations. Queue order is topological, so force-draining "through
        # the last needed unit" preserves all producer->consumer ordering.
        work = deque()

        def q_proj(n, mis):
            for mi in mis:
                for term in range(3):
                    work.append(
                        (("proj", n, mi), lambda mi=mi, n=n, tm=term: proj_unit(mi, n, tm))
                    )

        def q_v(ts):
            for t in ts:
                for term in range(3):
                    work.append((("v", t), lambda t=t, tm=term: v_unit(t, tm)))

        def drain(k=1):
            for _ in range(k):
                if work:
                    work.popleft()[1]()

        def drain_all():
            while work:
                work.popleft()[1]()


        # head h's scores need these packed q/k groups
        PROJ_GROUPS_FOR_HEAD = {0: (0, 1), 1: (0, 1), 2: (2,)}

        def force_drain_for(h, qt):
            """Emit queued units up to the last one attention(h, qt) depends on."""
            needed = set()
            for n in range(qt + 1):
                for mi in PROJ_GROUPS_FOR_HEAD[h]:
                    needed.add(("proj", n, mi))
            for t in range(4 * qt + 4):
                needed.add(("v", t))
            last = -1
            for i, (key, _) in enumerate(work):
                if key in needed:
                    last = i
            for _ in range(last + 1):
                work.popleft()[1]()

        def qh(h):
            col, off = [(0, 0), (0, 64), (1, 0)][h]
            return qT_sb[off : off + 64, col, :]

        def kh(h):
            col, off = [(0, 0), (0, 64), (1, 0)][h]
            return kT_sb[off : off + 64, col, :]

        zdst = [zT01[0:64, :], zT01[64:128, :], zT2[0:64, :]]

        # PV matmuls are pipelined a few pairs behind their exp across block
        # boundaries, so the in-order PE FIFO never waits on the exp/mask
        # chain, not even at the end of a block.
        pvq = deque()  # (block_serial, pv_closure)
        blk_serial = [0]

        def pv_drain(depth):
            while len(pvq) > depth:
                pvq.popleft()[1]()

        def pv_flush(upto_serial):
            while pvq and pvq[0][0] <= upto_serial:
                pvq.popleft()[1]()

        def attention(h, qt, per_pair, last=False):
            """scores^T -> exp -> causal mask -> PV into zp. Score matmuls for
            kt pairs land in one 2-bank psum tile so a single exp covers both;
            start=True zeroes the whole bank, so the unwritten low columns of
            narrowed diagonal halves exp to 1.0 and are never consumed."""
            zp = psz.tile([DH + 1, QT], F32)
            nkt = 4 * qt + 4
            blk = blk_serial[0]
            blk_serial[0] += 1

            def pv(kt, es_ap, lo):
                nc.tensor.matmul(
                    zp[:, lo:QT],
                    lhsT=v_aug[:, kt, h, :],
                    rhs=es_ap[:, lo:QT],
                    start=(kt == 0),
                    stop=(kt == nkt - 1),
                )

            def pv_q(kt, es_ap, lo):
                # last block: quarter q of zp sees its final write at
                # kt = nkt-4+q, so it can be normalized immediately after.
                # Only that closing kt is split off (multiple start=True
                # writes into one bank would re-zero siblings' columns).
                r = kt - (nkt - 4)
                if r < 0:
                    nc.tensor.matmul(
                        zp[:, 0:QT],
                        lhsT=v_aug[:, kt, h, :],
                        rhs=es_ap[:, 0:QT],
                        start=(kt == 0),
                        stop=False,
                        skip_group_check=True,
                    )
                else:
                    c0 = 128 * r
                    nc.tensor.matmul(
                        zp[:, c0 : c0 + 128],
                        lhsT=v_aug[:, kt, h, :],
                        rhs=es_ap[:, c0 : c0 + 128],
                        start=False,
                        stop=True,
                        skip_group_check=True,
                    )
                    if c0 + 128 < QT:
                        nc.tensor.matmul(
                            zp[:, c0 + 128 : QT],
                            lhsT=v_aug[:, kt, h, :],
                            rhs=es_ap[:, c0 + 128 : QT],
                            start=False,
                            stop=(kt == nkt - 1),
                            skip_group_check=True,
                        )
                    stairs(r)

            st_rec, st_bc = {}, {}

            def qcols(q):
                return slice(128 * q, 128 * (q + 1))

            def stairs(step):
                # stage skew keeps the in-order PE FIFO free of long waits:
                # each quarter's PE pieces (bc, o-proj) are emitted one or two
                # pv-steps after the pv that closed the quarter's psum group.
                if step >= 1:
                    st_bc[step - 1] = norm_bc(st_rec[step - 1], qcols(step - 1))
                st_rec[step] = norm_recip(zp, qcols(step))
                if step >= 1:
                    norm_post(zp, h, qt, st_bc[step - 1], qcols(step - 1))
                if step >= 2:
                    t = 4 * qt + step - 2
                    o_proj_unit(t, 0, evict_eng="act", tail=True)
                    o_proj_unit(t, 1, evict_eng="act", tail=True)
                drain(int(os.environ.get("KSTD", "2")))

            def stairs_final():
                st_bc[3] = norm_bc(st_rec[3], qcols(3))
                norm_post(zp, h, qt, st_bc[3], qcols(3))
                t = 4 * qt + 2
                o_proj_unit(t, 0, evict_eng="act", tail=True)
                o_proj_unit(t, 1, evict_eng="act", tail=True)
                t = 4 * qt + 3
                o_proj_unit(t, 0, evict_eng="dve", tail=True)
                o_proj_unit(t, 1, evict_eng="dve", tail=True)

            for kp in range(nkt // 2):
                kt0 = 2 * kp
                rr0 = kt0 - 4 * qt
                lo_pair = 128 * rr0 if rr0 > 0 else 0
                sp = ps2.tile([128, 2, QT], F32, tag="ps2")
                los = []
                for i in (0, 1):
                    kt = kt0 + i
                    rr = kt - 4 * qt
                    lo = 128 * rr if rr > 0 else 0
                    los.append(lo)
                    nc.tensor.matmul(
                        sp[:, i, lo:QT],
                        lhsT=kh(h)[:, kt * 128 : (kt + 1) * 128],
                        rhs=qh(h)[:, qt * QT + lo : (qt + 1) * QT],
                        start=True,
                        stop=True,
                    )
                es = expp.tile([128, 2, QT], F16, tag="expp")
                nc.scalar.activation(
                    out=es[:, :, lo_pair:QT], in_=sp[:, :, lo_pair:QT], func=AF.Exp
                )
                for i in (0, 1):
                    kt = kt0 + i
                    rr = kt - 4 * qt
                    lo = los[i]
                    if rr >= 0:  # diagonal square: zero where key > query
                        hi = min(lo + 128, QT)
                        nc.gpsimd.affine_select(
                            out=es[:, i, lo:hi],
                            in_=es[:, i, lo:hi],
                            compare_op=mybir.AluOpType.is_ge,
                            fill=0.0,
                            base=0,
                            channel_multiplier=-1,
                            pattern=[[1, hi - lo]],
                        )
                    fn = pv_q if last else pv
                    pvq.append(
                        (blk, lambda kt=kt, es=es, i=i, lo=lo, fn=fn: fn(kt, es[:, i], lo))
                    )
                drain(per_pair)
                depth = int(os.environ.get("KPVB", "8")) if qt >= 2 else int(os.environ.get("KPVD", "11"))
                if last:
                    depth = int(os.environ.get("KPVL", "8"))
                pv_drain(depth)
            if last:
                pv_flush(blk)
                stairs_final()
            return zp, blk

        def norm_recip(zp, cols):
            rec = small.tile([1, QT], F16, tag="rec")
            with nc.allow_low_precision(reason="fp16 normalize"):
                nc.vector.reciprocal(rec[:, cols], zp[DH : DH + 1, cols])
            return rec

        def norm_bc(rec, cols):
            bc_sb = small.tile([64, QT], F16, tag="bcsb")
            nc.gpsimd.partition_broadcast(bc_sb[:, cols], rec[0:1, cols])
            return bc_sb

        def norm_post(zp, h, qt, bc_sb, cols):
            nc.vector.tensor_mul(
                zdst[h][:, qt * QT : (qt + 1) * QT][:, cols],
                zp[0:DH, cols],
                bc_sb[:, cols],
            )

        def normalize(zp, h, qt, cols=slice(0, QT)):
            rec = norm_recip(zp, cols)
            bc_sb = norm_bc(rec, cols)
            norm_post(zp, h, qt, bc_sb, cols)

        # ---- schedule ----
        # prologue: only what attention(h0, qt0) needs, ordered by when each
        # term's operands land; the rest queues up.
        for term in range(3):
            proj_unit(0, 0, term=term)
            proj_unit(1, 0, term=term)
        for t in range(4):
            for term in range(3):
                v_unit(t, term)
        q_proj(0, mis=(2,))
        for n in range(1, NQT):
            q_proj(n, mis=(0, 1))
            q_v(range(4 * n, 4 * n + 2))
            q_proj(n, mis=(2,))
            q_v(range(4 * n + 2, 4 * n + 4))

        # qt2/qt3 blocks interleave so the exp-dense last q-tile spreads
        # over the whole back half instead of saturating Act at the end.
        BLOCKS = [(qt, h) for qt in range(NQT) for h in range(HPC)]

        deferred = []  # O-proj units held back to feed the exp-dense end

        def flush_pending(pending):
            pv_flush(pending[3])  # pending block's PV accumulation done
            normalize(*pending[:3])
            ph, pqt = pending[1], pending[2]
            if ph == HPC - 1:  # whole q-tile normalized -> O-proj ready
                for t in range(4 * pqt, 4 * pqt + 4):
                    for n2 in range(2):
                        unit = (("o", pqt), lambda t=t, n2=n2: o_proj_unit(t, n2))
                        (deferred if pqt <= 1 else work).append(unit)

        pending = None
        for qt, h in BLOCKS:
            per_pair = [int(c) for c in os.environ.get("KCAD", "4111")][qt]
            force_drain_for(h, qt)
            nsp = int(os.environ.get("KSPL", "4"))
            nsp2 = int(os.environ.get("KSP2", "4"))
            if (qt, h) == (3, 0):
                work.extend(deferred[:nsp])
                del deferred[:nsp]
            if (qt, h) == (3, 1):
                work.extend(deferred[: max(0, len(deferred) - nsp2)])
                del deferred[: max(0, len(deferred) - nsp2)]
            if (qt, h) == (3, 2):
                work.extend(deferred)
                deferred.clear()
            is_last = (qt, h) == BLOCKS[-1]
            if is_last:
                flush_pending(pending)
                pending = None
            zp, blk = attention(h, qt, per_pair, last=is_last)
            if pending is not None:
                flush_pending(pending)
            if not is_last:
                pending = (zp, h, qt, blk)
        drain_all()
    nc.finalize()
    return nc


_NC_CACHE = {}


def _f8(a):
    """main fp8 + residual fp8 (inputs pre-scaled x32, so both normal-range)."""
    import ml_dtypes

    f8 = ml_dtypes.float8_e4m3
    a = np.ascontiguousarray(a, np.float32)
    m = a.astype(f8)
    r = (a - m.astype(np.float32)).astype(f8)
    return np.ascontiguousarray(m), np.ascontiguousarray(r)


def make_in_maps(x, W_qkv, b_qkv, W_o):
    in_maps = []
    for c in range(8):
        b, g = divmod(c, 4)
        hs = [HPC * g + i for i in range(HPC)]
        qr = [np.arange(64 * h, 64 * h + 64) for h in hs]
        w_q = [W_qkv[i] * 0.125 for i in qr]
        w_k = [W_qkv[768 + i] for i in qr]
        w_v = [W_qkv[1536 + i] for i in qr]
        b_q = [b_qkv[i] * 0.125 for i in qr]
        b_k = [b_qkv[768 + i] for i in qr]
        # packed rows: m0=[q0 q1] m1=[k0 k1] m2=[q2 k2]
        wpk = np.concatenate(
            [w_q[0], w_q[1], w_k[0], w_k[1], w_q[2], w_k[2]], axis=0
        )
        wv = np.concatenate([w_v[0], w_v[1], w_v[2]], axis=0)
        bqk_col = np.zeros((128, 3), np.float32)
        bqk_col[:, 0] = np.concatenate([b_q[0], b_q[1]])
        bqk_col[:, 1] = np.concatenate([b_k[0], b_k[1]])
        bqk_col[:, 2] = np.concatenate([b_q[2], b_k[2]])
        # fp8 triples (values pre-scaled by 32; 1/1024 folded into evictions)
        xm = _f8(32.0 * x[b].T)
        wpm = _f8(32.0 * wpk.T)
        wvm = _f8(32.0 * wv.T)
        in_maps.append(
            {
                "x8": xm[0], "xr": xm[1],
                "wpk8": wpm[0], "wpkr": wpm[1],
                "wv8": wvm[0], "wvr": wvm[1],
                "woT": np.ascontiguousarray(
                    W_o[:, GD * g : GD * (g + 1)].T.astype(np.float16)
                ),
                "bqk": bqk_col,
                "vones": np.ones((128, 64), np.float16),
            }
        )
    return in_maps


def make_in_maps_for_test(inputs):
    return make_in_maps(
        np.asarray(inputs["x"], np.float32),
        np.asarray(inputs["W_qkv"], np.float32),
        np.asarray(inputs["b_qkv"], np.float32),
        np.asarray(inputs["W_o"], np.float32),
    )


def kernel(x, W_qkv, b_qkv, W_o, b_o):
    x = np.asarray(x, np.float32)
    W_qkv = np.asarray(W_qkv, np.float32)
    b_qkv = np.asarray(b_qkv, np.float32)
    W_o = np.asarray(W_o, np.float32)
    b_o = np.asarray(b_o, np.float32)

    if "nc" not in _NC_CACHE:
        _NC_CACHE["nc"] = build_bass()
    nc = _NC_CACHE["nc"]

    in_maps = make_in_maps(x, W_qkv, b_qkv, W_o)

    res = run_bass_kernel_spmd(
        nc,
        in_maps,
        list(range(8)),
        trace=bool(int(os.environ.get("KERNEL_TRACE", "0"))),
    )
    _NC_CACHE["last_results"] = res

    out = np.zeros((B, S, D), np.float32)
    for c in range(8):
        out[c // 4] += res.results[c]["out_p"].astype(np.float32)
    out += b_qkv[1536:] @ W_o.T + b_o
    return out

